# revision 100
# baseline (speedup 1.0000x reference)
"""BiEncoder (bidirectional LSTM over video features) Trainium2 kernel — v5.

Sharding: 8 NeuronCores = 8 batch groups (B=32 each); EACH core runs BOTH
directions over its 32 rows; no cross-core communication. The two per-core
scan chains (fwd/bwd) interleave on every engine to hide cross-engine
latency. Backward-direction consumption runs over v in reverse time order,
so the embedded sequence stays resident in SBUF and chunks are embedded in
the order {0,7},{1,6},{2,5},{3,4}.

v5 changes over v4 (255.7us -> 232.0us):
  - Embed in fp8 DoubleRow with bilinear residual compensation: video is
    host-quantized to an fp8 hi+residual pair (vt8, vtr8; scale 32) and
    W_e.T to an fp8 hi/lo pair (scale 16); three DR matmul sets
    (a@x8 + a@r8 + b@x8) give ~2^-8 accuracy at 3/4 the fp16 PE cost.
  - The x-projection (gate prefill) likewise: the embedded v is kept
    resident as an fp8 pair (x8sb + r8sb, scale 16, quantized on DVE from
    the f16 embed result), and W_ih as fp8 hi/lo DR pairs (scale 32);
    3 DR sets replace the fp16 matmuls (2048 -> 1536 cyc/step-dir).
  - Gate-bias PSUM init via an fp8 DoubleRow double-identity matmul over
    hi/lo bias planes (exact to ~2^-8, 256 cyc vs 512 for the fp16 ident).
  - Gate tanh split igf (chain) + o (emitted off-chain): the merged-16
    variant costs ~107ns extra on the recurrence chain.
  - LOOKAHEAD=1 (prefill one step ahead): with 6 pg banks the claimed bank
    was last read three steps back, absorbing the Tile scheduler's
    counting-semaphore rounding on the WAR edge (scheduler reorders by its
    own model; emission order is irrelevant, only structure matters).
  - hh matmuls unchanged: all 16 gate tiles fp8 DR off an fp8 h copy.
  - All gate tiles carry PSUM scale 512; x2-state cell update (C=2c, H=2h)
    in fused DVE scalar_tensor_tensor ops; gate tile order [i, g, f, o].
  - DMA tuning (transfers serialize globally at ~0.385ns/B/partition, so
    only byte-order on the critical path matters): wih8 lo-planes load
    after whh8; embed set order (a@x8, b@x8, a@r8) so a chunk's embed can
    start before its residual DMA lands; window outputs stream as
    quarter-window DMAs so the tail drains early; embed items paced to
    complete at tl==6 (any later and the window-end prefill -- which reads
    the pair being embedded -- would bind to stale data in Tile's
    program-order dependency tracking; any earlier wastes scan-critical
    PE slots).

Known constraints (hard-won):
  - Pool (gpsimd) cannot run TensorScalarPtr/elementwise on real HW
    (neuron ISA rejects the opcode) — only memset/iota/tensor_copy/DMA.
  - The steady phase is bound by the ~2.9us recurrence chain loop
    (mm_8 -> tanh -> m2/m1 -> c_new -> tanh_c -> h8 -> mm_8), not PE
    throughput; batch-half-split chains lose more to per-op fixed
    overheads and scheduler counting-sem conservatism than they gain.
"""

import sys
import time

for _p in ("/opt/trn_rl_repo", "/root/.axon_site/_ro/trn_rl_repo"):
    if _p not in sys.path:
        sys.path.insert(0, _p)

import numpy as np
import jax

try:
    jax.config.update("jax_compilation_cache_dir", "/tmp/jax_cc_cache")
    jax.config.update("jax_persistent_cache_min_entry_size_bytes", 0)
    jax.config.update("jax_persistent_cache_min_compile_time_secs", 0.0)
except Exception:
    pass

import concourse.tile as tile
from concourse import bacc, mybir
from concourse.bass import ts
from concourse.bass_utils import run_bass_kernel_spmd

F16 = mybir.dt.float16
F32 = mybir.dt.float32
F8 = mybir.dt.float8e4
F8_NP = mybir.dt.np(F8)
AF = mybir.ActivationFunctionType
OP = mybir.AluOpType
PM = mybir.MatmulPerfMode

B, T, F, P, H = 256, 64, 2048, 512, 512
NB = 8          # batch groups (one per core)
BC = B // NB    # 32 per-core batch
TC = 8
NCHUNK = T // TC
KF = F // 128
KP = P // 128
KH = H // 128
MG = 4 * H // 128  # 16 gate tiles: [i(0:4), g(4:8), f(8:12), o(12:16)]

IFO = list(range(16))  # ALL gate tiles fp8 for hh (incl. g: validated on
                       # the full batch against the true reference)
NI = len(IFO)

S_V = 16.0    # x8+r8 = fp8 pair of S_V * (v + b_e)
S_VID = 32.0  # vt8+vtr8 = fp8 pair of S_VID * video
S_WE = 16.0   # wet8a/b = fp8 pair of S_WE * W_e.T
S_H = 16.0    # h8 = fp8(S_H * H), H = 2h
S_WH = 32.0   # whh8 = fp8(S_WH * Whh_eff)
S_G = S_H * S_WH  # 512; uniform gate PSUM scale

# embed order: window w consumes chunks w (fwd) and 7-w (bwd)
EMBED_ORDER = [0, 7, 1, 6, 2, 5, 3, 4]


def build_nc():
    nc = bacc.Bacc("TRN2", target_bir_lowering=False, debug=False, num_devices=8)

    vt8_d = nc.dram_tensor("vt8", [NCHUNK, 128, KF, TC, BC], F8,
                           kind="ExternalInput")
    vtr8_d = nc.dram_tensor("vtr8", [NCHUNK, 128, KF, TC, BC], F8,
                            kind="ExternalInput")
    # embed weights as fp8 hi/lo DoubleRow pairs (K-tile pairs on dim1)
    wet8_d = [nc.dram_tensor(f"w_et8{ab}", [128, KF // 2, 2, P], F8,
                             kind="ExternalInput") for ab in "ab"]
    bet_d = nc.dram_tensor("b_e_t", [128, KP], F32, kind="ExternalInput")
    # x-projection weights as fp8 hi/lo DoubleRow pairs (bilinear
    # compensation: W8a@x8 + W8a@r8 + W8b@x8 ~ W@x to ~2^-8)
    wih8_d = [
        [nc.dram_tensor(f"w_ih8{ab}{d}", [128, 2, MG, 2, 128], F8,
                        kind="ExternalInput") for ab in "ab"]
        for d in range(2)
    ]
    whh8_d = [nc.dram_tensor(f"w_hh8{d}", [128, 2, NI, 2, 128], F8,
                             kind="ExternalInput") for d in range(2)]
    # fp8 hi/lo bias planes consumed by a DoubleRow double-identity matmul
    # (exact to ~2^-8): out[p,m,b] = sum_k I[k,p]*(hi[k,m]+lo[k,m])
    biasbc_d = [nc.dram_tensor(f"biasbc{d}", [128, 2, MG, BC], F8,
                               kind="ExternalInput") for d in range(2)]
    ident8_d = nc.dram_tensor("ident8", [128, 2, 128], F8, kind="ExternalInput")
    out_d = nc.dram_tensor("out_h", [2, NCHUNK, 128, TC, KH, BC], F16,
                           kind="ExternalOutput")

    with tile.TileContext(nc) as tc:
        with (
            tc.tile_pool(name="const", bufs=1) as const,
            tc.tile_pool(name="vload", bufs=2) as vload,
            tc.tile_pool(name="state", bufs=4) as state,
            tc.tile_pool(name="tmp", bufs=5) as tmp,
            tc.tile_pool(name="psv", bufs=2, space="PSUM") as psv,
            tc.tile_pool(name="psg", bufs=6, space="PSUM") as psg,
        ):
            # embed-critical consts first; the big scan weights are DMA'd
            # after the first two video chunks (below) so the prologue
            # embeds aren't stuck behind ~10MB of weight traffic.
            wet8 = []
            for ab in range(2):
                w = const.tile([128, KF // 2, 2, P], F8, name=f"wet8{ab}")
                nc.sync.dma_start(w[:], wet8_d[ab].ap())
                wet8.append(w)
            bet = const.tile([128, KP], F32)
            nc.sync.dma_start(bet[:], bet_d.ap())
            ident8 = const.tile([128, 2, 128], F8)
            nc.sync.dma_start(ident8[:], ident8_d.ap())
            wihg, whh8, biasbc = [], [], []

            def load_scan_weights():
                # prefill consumers (wihg, biasbc) first, recurrence weights
                # (whhg, whh8) after — they are needed ~2us later
                # hi planes + bias + whh first (the scan can start on
                # them); the lo planes (3rd prefill matmul set) ride last
                for d in range(2):
                    ws = [None, None]
                    w1 = const.tile([128, 2, MG, 2, 128], F8, name=f"wih8a{d}")
                    nc.sync.dma_start(w1[:], wih8_d[d][0].ap())
                    ws[0] = w1
                    wihg.append(ws)
                    w4 = const.tile([128, 2, MG, BC], F8, name=f"biasbc{d}")
                    nc.sync.dma_start(w4[:], biasbc_d[d].ap())
                    biasbc.append(w4)
                for d in range(2):
                    w3 = const.tile([128, 2, NI, 2, 128], F8, name=f"whh8{d}")
                    nc.sync.dma_start(w3[:], whh8_d[d].ap())
                    whh8.append(w3)
                for d in range(2):
                    w2 = const.tile([128, 2, MG, 2, 128], F8, name=f"wih8b{d}")
                    nc.sync.dma_start(w2[:], wih8_d[d][1].ap())
                    wihg[d][1] = w2

            # whole embedded sequence stays resident (bwd reads it
            # reversed) as an fp8 pair: x8 = fp8(16*v), r8 = fp8(16*v - x8)
            x8sb = const.tile([128, KP, T, BC], F8)
            r8sb = const.tile([128, KP, T, BC], F8)

            h_prev, h8_prev, c_prev = [None, None], [], []
            for d in range(2):
                h8p = state.tile([128, KH, BC], F8, tag=f"h8{d}")
                nc.gpsimd.memset(h8p[:], 0.0)
                cp = state.tile([128, KH, BC], F16, tag=f"c{d}")
                nc.gpsimd.memset(cp[:], 0.0)
                h8_prev.append(h8p)
                c_prev.append(cp)

            def phase_a_items(c):
                """Embed chunk c into the resident x8/r8 fp8 tiles."""
                vch8 = vload.tile([128, KF, TC * BC], F8, tag="vch8")
                vchr8 = vload.tile([128, KF, TC * BC], F8, tag="vchr8")

                def dma_item():
                    nc.sync.dma_start(
                        vch8[:], vt8_d.ap()[c].rearrange("p ko t b -> p ko (t b)")
                    )
                    nc.sync.dma_start(
                        vchr8[:],
                        vtr8_d.ap()[c].rearrange("p ko t b -> p ko (t b)"),
                    )

                def embed_item(mp):
                    pv = psv.tile([128, TC * BC], F32, tag="pv")
                    # residual set last: the chunk's vtr8 DMA is only
                    # needed by the 3rd matmul set
                    sets = ((0, vch8), (1, vch8), (0, vchr8))
                    for si, (wti, vsrc) in enumerate(sets):
                        for pr in range(KF // 2):
                            nc.tensor.matmul(
                                pv[:],
                                wet8[wti][:, pr, :, ts(mp, 128)],
                                vsrc[:, 2 * pr : 2 * pr + 2, :],
                                start=(si == 0 and pr == 0),
                                stop=(si == 2 and pr == KF // 2 - 1),
                                perf_mode=PM.DoubleRow,
                            )
                    # v16 = 16*(v + b_e); then split into fp8 hi (x8) and
                    # residual (r8) for the DoubleRow x-projection
                    v16 = tmp.tile([128, TC * BC], F16, tag="v16", name="v16")
                    nc.scalar.activation(
                        v16[:], pv[:], AF.Identity,
                        bias=bet[:, mp : mp + 1], scale=S_V / (S_VID * S_WE),
                    )
                    x8t = x8sb[:, mp, c * TC : (c + 1) * TC, :].rearrange(
                        "p t b -> p (t b)")
                    nc.vector.tensor_scalar(x8t, v16[:], 1.0, None, OP.mult)
                    nc.vector.scalar_tensor_tensor(
                        r8sb[:, mp, c * TC : (c + 1) * TC, :].rearrange(
                            "p t b -> p (t b)"),
                        x8t, -1.0, v16[:], OP.mult, OP.add,
                    )

                return [dma_item] + [lambda mp=mp: embed_item(mp) for mp in range(KP)]

            def emit_gate_prefill(d, pg, t):
                # v-time: fwd consumes t, bwd consumes T-1-t
                vt_time = t if d == 0 else T - 1 - t
                # bias init: fp8 DoubleRow double-identity matmul (256
                # cyc, half the fp16 ident cost; hi+lo planes keep it exact
                # to ~2^-8)
                nc.tensor.matmul(
                    pg[:, :, :],
                    ident8[:, :, :],
                    biasbc[d][:, :, :, :],
                    start=True,
                    stop=False,
                    perf_mode=PM.DoubleRow,
                    skip_group_check=True,
                )
                for src8, wti in (
                    (x8sb, 0),  # W8a @ x8
                    (r8sb, 0),  # W8a @ r8
                    (x8sb, 1),  # W8b @ x8
                ):
                    for pr in range(2):
                        for m in range(MG):
                            nc.tensor.matmul(
                                pg[:, m, :],
                                wihg[d][wti][:, pr, m, :, :],
                                src8[:, 2 * pr : 2 * pr + 2, vt_time, :],
                                start=False,
                                stop=False,
                                perf_mode=PM.DoubleRow,
                                skip_group_check=True,
                            )

            def scan_step(d, pg, tl, hstage):
                """Generator: yields at engine-stage boundaries so the two
                directions' same-stage ops can be emitted adjacently."""
                th = tmp.tile([128, MG, BC], F16, tag=f"th{d}")

                def mm_8(tiles):
                    for pr in range(2):
                        for m in tiles:
                            mi = IFO.index(m)
                            nc.tensor.matmul(
                                pg[:, m, :],
                                whh8[d][:, pr, mi, :, :],
                                h8_prev[d][:, 2 * pr : 2 * pr + 2, :],
                                start=False,
                                stop=False,
                                perf_mode=PM.DoubleRow,
                                skip_group_check=True,
                            )

                mm_8(range(MG))  # all gates fp8 DoubleRow
                yield
                nc.scalar.activation(
                    th[:, 0:12, :], pg[:, 0:12, :], AF.Tanh, scale=1.0 / S_G
                )
                m2 = tmp.tile([128, KH, BC], F16, tag=f"m2{d}")
                nc.vector.scalar_tensor_tensor(
                    m2[:], th[:, 0:4, :], 1.0, th[:, 4:8, :], OP.add, OP.mult
                )
                m1 = tmp.tile([128, KH, BC], F16, tag=f"m1{d}")
                nc.vector.scalar_tensor_tensor(
                    m1[:], th[:, 8:12, :], 1.0, c_prev[d][:], OP.add, OP.mult
                )
                yield
                c_new = state.tile([128, KH, BC], F16, tag=f"c{d}")
                nc.vector.scalar_tensor_tensor(
                    c_new[:], m1[:], 0.5, m2[:], OP.mult, OP.add
                )
                yield
                nc.scalar.activation(
                    th[:, 12:16, :], pg[:, 12:16, :], AF.Tanh, scale=1.0 / S_G
                )
                tc_t = tmp.tile([128, KH, BC], F16, tag=f"tct{d}")
                nc.scalar.activation(tc_t[:], c_new[:], AF.Tanh, scale=0.5)
                yield
                h8_new = state.tile([128, KH, BC], F8, tag=f"h8{d}")
                for hv in range(2):
                    sl = slice(2 * hv, 2 * hv + 2)
                    nc.vector.scalar_tensor_tensor(
                        h8_new[:, sl, :], th[:, 12 + 2 * hv : 14 + 2 * hv, :],
                        1.0, tc_t[:, sl, :], OP.add, OP.mult,
                    )
                h_new = hstage[:, tl, :, :]
                nc.vector.scalar_tensor_tensor(
                    h_new, th[:, 12:16, :], 1.0, tc_t[:], OP.add, OP.mult
                )
                h_prev[d], h8_prev[d], c_prev[d] = h_new, h8_new, c_new

            # ---- emission ----
            # LOOKAHEAD=1 with 6 pg banks: a claimed bank was last read
            # three steps back — slack that absorbs the scheduler's
            # counting-semaphore rounding on the WAR edge.
            LOOKAHEAD = 1
            from collections import deque

            # prologue: embed chunks 0 and 7, then prefills for steps 0,1.
            # Video DMAs go first, then the scan weights, then the embeds.
            pro0 = phase_a_items(EMBED_ORDER[0])
            pro1 = phase_a_items(EMBED_ORDER[1])
            pro0[0]()
            pro1[0]()
            load_scan_weights()
            for it in pro0[1:] + pro1[1:]:
                it()
            pg_q = [deque(), deque()]
            for glob in range(LOOKAHEAD):
                for d in range(2):
                    pg = psg.tile([128, MG, BC], F32, tag="pg")
                    emit_gate_prefill(d, pg, glob)
                    pg_q[d].append(pg)

            items, n_items, emitted = [], 0, 0
            hstages = [None, None]
            for glob in range(T):
                c, tl = divmod(glob, TC)
                if tl == 0:
                    for d in range(2):
                        hst = state.tile(
                            [128, TC, KH, BC], F16, tag=f"hs{d}", name=f"hs{d}"
                        )
                        hstages[d] = hst
                    # embed the pair for window c+1 during window c
                    pos = 2 * (c + 1)
                    items = []
                    if pos < NCHUNK:
                        items = phase_a_items(EMBED_ORDER[pos])
                        items += phase_a_items(EMBED_ORDER[pos + 1])
                    n_items, emitted = len(items), 0
                gens = [
                    scan_step(d, pg_q[d].popleft(), tl, hstages[d])
                    for d in range(2)
                ]
                live = list(gens)
                while live:
                    live = [g for g in live if next(g, StopIteration) is None]
                if tl == TC - 1 and n_items:
                    # drain the pair's remaining embed items BEFORE the
                    # window-end prefill (which reads the pair's data)
                    while emitted < n_items:
                        items[emitted]()
                        emitted += 1
                nxt = glob + LOOKAHEAD
                if nxt < T:
                    for d in range(2):
                        pg = psg.tile([128, MG, BC], F32, tag="pg")
                        emit_gate_prefill(d, pg, nxt)
                        pg_q[d].append(pg)
                if n_items:
                    # pace embeds across the window (complete by tl==6;
                    # the tl==TC-1 pre-prefill drain above is the safety
                    # net for the window-end prefill's data dependence)
                    want = min(n_items, (n_items * (tl + 1)) // (TC - 1))
                    while emitted < want:
                        items[emitted]()
                        emitted += 1
                if tl == TC - 1:
                    # quarter-window DMAs: earlier rows are long complete
                    # (region deps) and stream out while the last steps still
                    # run — trims the end-of-kernel drain
                    for d in range(2):
                        for q in range(4):
                            nc.sync.dma_start(
                                out_d.ap()[d, c][:, 2 * q : 2 * q + 2],
                                hstages[d][:, 2 * q : 2 * q + 2],
                            )

    nc.compile()
    return nc


_CACHED_NC = None


def _get_nc():
    global _CACHED_NC
    if _CACHED_NC is None:
        _CACHED_NC = build_nc()
    return _CACHED_NC


def _prep_inputs(video_feats, W_e, b_e, W_ih1, W_hh1, b_ih1, b_hh1,
                 W_ih2, W_hh2, b_ih2, b_hh2):
    # gate row scaling (sigmoid-via-tanh): i,f,o rows 0.5; g rows 1.0
    s = np.ones((4 * H,), np.float32)
    s[0 * H : 2 * H] = 0.5
    s[3 * H : 4 * H] = 0.5
    perm = np.concatenate(
        [
            np.arange(0 * H, 1 * H),  # i
            np.arange(2 * H, 3 * H),  # g
            np.arange(1 * H, 2 * H),  # f
            np.arange(3 * H, 4 * H),  # o
        ]
    )

    wet_s = W_e.T.astype(np.float32) * S_WE        # [F, P]
    wet_hi = wet_s.astype(F8_NP)
    wet_lo = (wet_s - wet_hi.astype(np.float32)).astype(F8_NP)
    wet8 = [
        np.ascontiguousarray(
            w.reshape(KF // 2, 2, 128, P).transpose(2, 0, 1, 3)
        )
        for w in (wet_hi, wet_lo)
    ]
    bet = np.ascontiguousarray(b_e.reshape(KP, 128).T).astype(np.float32) * S_V

    def pack8(Wt, scale):
        Wq = (Wt * scale).astype(F8_NP)
        out = np.zeros((128, 2, NI, 2, 128), F8_NP)
        for mi, m in enumerate(IFO):
            blk = Wq[:, m * 128 : (m + 1) * 128]
            for pr in range(Wt.shape[0] // 256):
                for j in range(2):
                    kt = 2 * pr + j
                    out[:, pr, mi, j, :] = blk[kt * 128 : (kt + 1) * 128, :]
        return np.ascontiguousarray(out)

    per_dir = []
    for (W_ih, W_hh, b_ih, b_hh) in (
        (W_ih1, W_hh1, b_ih1, b_hh1),
        (W_ih2, W_hh2, b_ih2, b_hh2),
    ):
        wih_eff = ((W_ih * s[:, None])[perm]).T.astype(np.float32)
        whh_eff = ((W_hh * s[:, None] * 0.5)[perm]).T.astype(np.float32)
        bb = (((b_ih + b_hh) * s)[perm]).astype(np.float32)
        bbt = np.ascontiguousarray(bb.reshape(MG, 128).T) * S_G  # [128, MG]
        b_hi = bbt.astype(F8_NP)
        b_lo = (bbt - b_hi.astype(np.float32)).astype(F8_NP)
        bias2 = np.stack([b_hi, b_lo], axis=1)  # [128, 2, MG]
        biasbc = np.broadcast_to(bias2[:, :, :, None], (128, 2, MG, BC))
        wih_s = wih_eff * (S_G / S_V)  # [P, 4H], fp8 pair at scale 32
        wih_hi = wih_s.astype(F8_NP)
        wih_lo = (wih_s - wih_hi.astype(np.float32)).astype(F8_NP)
        wih8 = [pack8(w, 1.0) for w in (wih_hi.astype(np.float32),
                                        wih_lo.astype(np.float32))]
        whh8 = pack8(whh_eff, S_G)
        per_dir.append((wih8, whh8, np.ascontiguousarray(biasbc)))

    vt_s = np.ascontiguousarray(video_feats.transpose(2, 1, 0)) * S_VID
    vt8_full = vt_s.astype(F8_NP)
    vtr8_full = (vt_s - vt8_full.astype(np.float32)).astype(F8_NP)

    in_maps = []
    for core in range(8):
        sl = slice(core * BC, (core + 1) * BC)
        vt8 = np.ascontiguousarray(
            vt8_full[:, :, sl].reshape(KF, 128, NCHUNK, TC, BC)
            .transpose(2, 1, 0, 3, 4)
        )
        vtr8 = np.ascontiguousarray(
            vtr8_full[:, :, sl].reshape(KF, 128, NCHUNK, TC, BC)
            .transpose(2, 1, 0, 3, 4)
        )
        eye8 = np.eye(128, dtype=F8_NP)
        im = {
            "vt8": vt8,
            "vtr8": vtr8,
            "w_et8a": wet8[0],
            "w_et8b": wet8[1],
            "b_e_t": bet,
            "ident8": np.ascontiguousarray(
                np.broadcast_to(eye8[:, None, :], (128, 2, 128))
            ),
        }
        for d in range(2):
            wih8, whh8, biasbc = per_dir[d]
            im[f"w_ih8a{d}"] = wih8[0]
            im[f"w_ih8b{d}"] = wih8[1]
            im[f"w_hh8{d}"] = whh8
            im[f"biasbc{d}"] = biasbc
        in_maps.append(im)
    return in_maps


last_exec_ns = None
last_wall_s = None


def kernel(**inputs):
    global last_exec_ns, last_wall_s
    nc = _get_nc()
    inputs = {k: np.asarray(v, dtype=np.float32) for k, v in inputs.items()}
    in_maps = _prep_inputs(**inputs)
    t0 = time.perf_counter()
    res = run_bass_kernel_spmd(nc, in_maps, core_ids=list(range(8)))
    last_wall_s = time.perf_counter() - t0
    last_exec_ns = res.exec_time_ns

    lstm1 = np.empty((B, T, H), np.float32)
    lstm2 = np.empty((B, T, H), np.float32)
    for core in range(8):
        oh = res.results[core]["out_h"]  # [2, NCHUNK, 128, TC, KH, BC], holds 2h
        for d, dst in ((0, lstm1), (1, lstm2)):
            h = np.transpose(
                oh[d].astype(np.float32), (4, 0, 2, 3, 1)
            ).reshape(BC, T, H)
            h *= 0.5
            if d == 1:
                h = h[:, ::-1, :]
            dst[core * BC : (core + 1) * BC] = h
    return (lstm1, lstm2)

